# revision 13
# baseline (speedup 1.0000x reference)
"""MHGCN kernel for 8 Trainium2 NeuronCores.

Strategy: row-shard the [7,4096,4096] A_stack across 8 cores (512 rows each,
read once ~29.4MB/core bf16).  Each core streams its strip and builds the
transposed row-block of final_A:
    FT[k, m] = merged[m, k] + merged[k, m] + s*tanh_interaction[m, k]
 - local terms (merged[m,k], interaction) computed elementwise, balanced
   across DVE / ACT / GPSIMD using 4x-mode tensor_scalar ops + TT trees.
 - merged[k, m] column blocks arrive via FOUR sub-AllToAlls (one per local
   row-tile), fired as each row-tile finishes so 3 of 4 overlap phase 1.
Interaction identity:  arg = sum_i Q_i * E_i,  Q_i = 1.5 R_i + 1(R_i>0),
E_i = sum_{j!=i} 0.4 M_ij R_j;  interaction = tanh(arg).
Both GCN layers are then FT^T @ Y matmuls; V=final_A@G is AllGathered
between layers.  struct_adj = (encode*sw) @ encode^T is rank-7 and computed
locally without materializing it.
"""
import sys

sys.path.insert(0, "/opt/trn_rl_repo")

import numpy as np

import bass_rust
import concourse.bass as bass
import concourse.tile as tile
from concourse import mybir
from concourse.bass_utils import run_bass_kernel_spmd
from concourse.masks import make_identity
from concourse.vector_clock import ScopedClock

F32 = mybir.dt.float32
BF16 = mybir.dt.bfloat16
AF = mybir.ActivationFunctionType
OP = mybir.AluOpType

P = 128
N = 4096
NFEAT = 128
OUT = 64
NREL = 7
NCORES = 8
ROWS = N // NCORES        # 512 rows per core
NT = ROWS // P            # 4 row tiles per core
KT = N // P               # 32 k tiles
C = 1024                  # streaming column chunk
NCH = N // C              # 4 chunks per row tile
DST = ROWS                # exchange block width (512)


def _patched_drain_and_barrier(self, tick_clock, wait_clock):
    # Stock Tile attaches every outstanding proc's sem wait to one Drain;
    # this walrus build caps sync waits per instruction, so split them
    # into single-wait drains.
    drain_inst = self.nc.sync.drain()
    wait_clock.add_sem_waits(
        drain_inst.ins, ScopedClock({None: tick_clock.global_clock})
    )
    si = drain_inst.ins.sync_info
    if si is not None and len(si.on_wait) > 1:
        waits = list(si.on_wait)
        si.on_wait = [waits[0]]
        for w in waits[1:]:
            extra = self.nc.sync.drain()
            extra.ins.sync_info = bass_rust.SyncInfo(on_wait=[w], on_update=[])
    self.nc.all_engine_barrier()
    assert self.sems is not None
    popped = self.nc._tile_sem_poison_stack.pop()
    assert popped is self._sem_poison
    self.nc.clear_and_free_semaphores(list(self.sems.allocated().values()))
    self.nc.all_engine_barrier()


tile.TileContext._drain_and_barrier = _patched_drain_and_barrier


def _split_multi_waits(nc, limit=1):
    """Walrus in this container caps sync-wait commands per instruction.
    Hoist all-but-`limit` waits of any instruction onto single-wait NoOps
    inserted just before it on the same engine queue."""
    cnt = 0
    for fn in nc.m.functions:
        for blk in fn.blocks:
            lst = list(blk.instructions)
            out = []
            changed = False
            for inst in lst:
                si = inst.sync_info
                if si is not None and len(si.on_wait) > limit:
                    waits = list(si.on_wait)
                    for w in waits[:-limit]:
                        n = bass_rust.InstNoOp(name=f"wsplit-{cnt}")
                        cnt += 1
                        n.engine = inst.engine
                        n.bass_nofuse = True
                        n.sync_info = bass_rust.SyncInfo(on_wait=[w],
                                                         on_update=[])
                        nc.register_instruction(n, overwrite=True)
                        out.append(n)
                    si.on_wait = waits[-limit:]
                    changed = True
                out.append(inst)
            if changed:
                blk.instructions = out
    return cnt


def _normalize(nc, pool, psum, x, out_dram, i):
    """l2-normalize rows of x [P, OUT] and DMA to out_dram[i*P:(i+1)*P]."""
    sq = pool.tile([P, OUT], F32, tag="nrm_sq")
    nrm = pool.tile([P, 1], F32, tag="nrm_n")
    nc.vector.tensor_tensor(sq[:], x[:], x[:], OP.mult)
    nc.vector.tensor_reduce(nrm[:], sq[:], mybir.AxisListType.X, OP.add)
    nr = pool.tile([P, 1], F32, tag="nrm_r")
    nc.scalar.activation(nr[:], nrm[:], AF.Sqrt)
    nc.vector.tensor_scalar(nr[:], nr[:], 1e-12, None, OP.max)
    ninv = pool.tile([P, 1], F32, tag="nrm_i")
    nc.vector.reciprocal(ninv[:], nr[:])
    y = pool.tile([P, OUT], F32, tag="nrm_y")
    nc.vector.tensor_scalar(y[:], x[:], ninv[:], None, OP.mult)
    nc.sync.dma_start(out=out_dram[i * P:(i + 1) * P, :], in_=y[:])


def build_nc():
    nc = bass.Bass()

    a_strip = nc.dram_tensor("a_strip", [NREL, ROWS, N], BF16, kind="ExternalInput")
    featT = nc.dram_tensor("featT", [NFEAT, N], F32, kind="ExternalInput")
    encode = nc.dram_tensor("encode", [N, NREL], F32, kind="ExternalInput")
    enc_rows = nc.dram_tensor("enc_rows", [ROWS, NREL], F32, kind="ExternalInput")
    W1 = nc.dram_tensor("W1", [NFEAT, OUT], F32, kind="ExternalInput")
    W2 = nc.dram_tensor("W2", [OUT, OUT], F32, kind="ExternalInput")
    b1 = nc.dram_tensor("b1", [1, OUT], F32, kind="ExternalInput")
    b2 = nc.dram_tensor("b2", [1, OUT], F32, kind="ExternalInput")
    wb = nc.dram_tensor("wb", [1, NREL], F32, kind="ExternalInput")
    ri = nc.dram_tensor("ri", [1, 9], F32, kind="ExternalInput")
    s_ = nc.dram_tensor("s_", [1, 1], F32, kind="ExternalInput")
    sw = nc.dram_tensor("sw", [NREL, 1], F32, kind="ExternalInput")

    o_res = nc.dram_tensor("o_res", [ROWS, OUT], F32, kind="ExternalOutput")
    o_b1 = nc.dram_tensor("o_b1", [ROWS, OUT], F32, kind="ExternalOutput")
    o_b2 = nc.dram_tensor("o_b2", [ROWS, OUT], F32, kind="ExternalOutput")

    groups = [list(range(NCORES))]

    with tile.TileContext(nc) as tc:
        with (
            tc.tile_pool(name="persist", bufs=1) as pp,
            tc.tile_pool(name="dram", bufs=1, space="DRAM") as dpool,
        ):
            # ---- constants / small tensors ----
            ident = pp.tile([P, P], F32)
            make_identity(nc, ident)
            identb = pp.tile([P, P], BF16)
            nc.vector.tensor_copy(identb[:], ident[:])

            ones_1p = pp.tile([1, P], F32)
            nc.vector.memset(ones_1p[:], 1.0)

            # scalar staging: [0:7]=w_r, [7:16]=M flat, [16]=s
            sstage = pp.tile([1, 17], F32)
            nc.sync.dma_start(out=sstage[:, 0:NREL], in_=wb[:])
            nc.sync.dma_start(out=sstage[:, NREL:NREL + 9], in_=ri[:])
            nc.sync.dma_start(out=sstage[:, 16:17], in_=s_[:])

            W1t = pp.tile([NFEAT, OUT], F32)
            nc.sync.dma_start(out=W1t[:], in_=W1[:])
            W2t = pp.tile([OUT, OUT], F32)
            nc.sync.dma_start(out=W2t[:], in_=W2[:])
            b1st = pp.tile([1, OUT], F32)
            nc.sync.dma_start(out=b1st[:], in_=b1[:])
            b2st = pp.tile([1, OUT], F32)
            nc.sync.dma_start(out=b2st[:], in_=b2[:])
            swt = pp.tile([NREL, 1], F32)
            nc.sync.dma_start(out=swt[:], in_=sw[:])

            scal = pp.tile([P, 17], F32)
            b1b = pp.tile([P, OUT], F32)
            b2b = pp.tile([P, OUT], F32)
            with tc.tile_pool(name="ppsum", bufs=1, space="PSUM") as pps:
                pb = pps.tile([P, 17], F32, tag="pb")
                nc.tensor.matmul(pb[:], lhsT=ones_1p[:], rhs=sstage[:],
                                 start=True, stop=True)
                nc.vector.tensor_copy(scal[:], pb[:])
                pb1 = pps.tile([P, OUT], F32, tag="pb1")
                nc.tensor.matmul(pb1[:], lhsT=ones_1p[:], rhs=b1st[:],
                                 start=True, stop=True)
                nc.vector.tensor_copy(b1b[:], pb1[:])
                pb2 = pps.tile([P, OUT], F32, tag="pb2")
                nc.tensor.matmul(pb2[:], lhsT=ones_1p[:], rhs=b2st[:],
                                 start=True, stop=True)
                nc.vector.tensor_copy(b2b[:], pb2[:])

            # fp32 broadcast scalars (TensorScalarPtr requires fp32 scalars)
            scal04 = pp.tile([P, 9], F32)
            nc.vector.tensor_scalar(scal04[:], scal[:, NREL:NREL + 9], 0.4,
                                    None, OP.mult)
            # s-scaled identity: FT transpose-accumulate applies s*tanh via rhs
            sidentb = pp.tile([P, P], BF16)
            nc.vector.tensor_scalar(sidentb[:], identb[:], scal[:, 16:17],
                                    None, OP.mult)

            def w_ap(r):
                return scal[:, r:r + 1]

            s_ap = scal[:, 16:17]

            def c04_ap(i, j):
                return scal04[:, 3 * i + j:3 * i + j + 1]

            # ---- persistent big tensors (bf16; PSUM accumulates fp32) ----
            FT = pp.tile([P, KT * ROWS], BF16)    # final_A^T: 32 k-tiles x [128, 512]
            YG = pp.tile([P, KT * 2 * OUT], BF16)  # [Y1 | G] per k-tile

            # ---- DRAM bounce buffers (4 sub-exchanges, one per row tile) ----
            sendbuf = dpool.tile([NT, NCORES * P, DST], BF16)
            recvbuf = dpool.tile([NT, NCORES * P, DST], BF16)
            agin = dpool.tile([ROWS, OUT], BF16)
            agout = dpool.tile([N, OUT], BF16, addr_space="Shared")

            # ---- prep: Y1 = feature @ W1 ----
            with (
                tc.tile_pool(name="prep", bufs=1) as prep,
                tc.tile_pool(name="preppsum", bufs=2, space="PSUM") as prps,
            ):
                ftile = prep.tile([NFEAT, N], F32)
                nc.sync.dma_start(out=ftile[:], in_=featT[:])
                fbf = prep.tile([NFEAT, N], BF16)
                nc.vector.tensor_copy(fbf[:], ftile[:])
                W1b = pp.tile([NFEAT, OUT], BF16)
                nc.vector.tensor_copy(W1b[:], W1t[:])
                W2b = pp.tile([OUT, OUT], BF16)
                nc.vector.tensor_copy(W2b[:], W2t[:])
                # W12 = W1 @ W2 (via W1^T transpose), h = b1 @ W2
                pw1t = prps.tile([P, P], BF16, tag="prsm")
                nc.tensor.transpose(pw1t[:OUT, :NFEAT], W1b[:], identb[:])
                W1T = prep.tile([OUT, NFEAT], BF16)
                nc.vector.tensor_copy(W1T[:], pw1t[:OUT, :NFEAT])
                pw12 = prps.tile([NFEAT, OUT], F32, tag="prsm")
                nc.tensor.matmul(pw12[:], lhsT=W1T[:], rhs=W2b[:],
                                 start=True, stop=True)
                W12b = pp.tile([NFEAT, OUT], BF16)
                nc.vector.tensor_copy(W12b[:], pw12[:])
                b1v = prep.tile([OUT, 1], BF16)
                pb1t = prps.tile([OUT, 1], BF16, tag="prsm")
                b1bf = prep.tile([1, OUT], BF16)
                nc.vector.tensor_copy(b1bf[:], b1st[:])
                nc.tensor.transpose(pb1t[:], b1bf[:], identb[:1, :1])
                nc.vector.tensor_copy(b1v[:], pb1t[:])
                phh = prps.tile([1, OUT], F32, tag="prsm")
                nc.tensor.matmul(phh[:], lhsT=b1v[:], rhs=W2b[:],
                                 start=True, stop=True)
                hst = prep.tile([1, OUT], F32)
                nc.vector.tensor_copy(hst[:], phh[:])
                phb = prps.tile([P, OUT], F32, tag="prsm")
                nc.tensor.matmul(phb[:], lhsT=ones_1p[:], rhs=hst[:],
                                 start=True, stop=True)
                hb = pp.tile([P, OUT], F32)
                nc.vector.tensor_copy(hb[:], phb[:])
                hbb = pp.tile([P, OUT], BF16)
                nc.vector.tensor_copy(hbb[:], hb[:])

                for kt in range(KT):
                    pm = prps.tile([P, OUT], F32, tag="y1p")
                    nc.tensor.matmul(pm[:], lhsT=fbf[:, kt * P:(kt + 1) * P],
                                     rhs=W1b[:], start=True, stop=True)
                    nc.vector.tensor_copy(
                        YG[:, kt * 2 * OUT:kt * 2 * OUT + OUT], pm[:])
                    pg_ = prps.tile([P, OUT], F32, tag="gp")
                    nc.tensor.matmul(pg_[:], lhsT=fbf[:, kt * P:(kt + 1) * P],
                                     rhs=W12b[:], start=True, stop=True)
                    nc.vector.tensor_copy(
                        YG[:, kt * 2 * OUT + OUT:(kt + 1) * 2 * OUT], pg_[:])

            # ---- phase 1: stream A row block ----
            # Per chunk [128, C]: R_j = relation j slice.
            #   g_j = 1(R_j > 0)                    DVE tensor_scalar is_gt (4x)
            #   Q_j = 1.5 R_j + g_j                 (Q2 on GPSIMD via STT)
            #   E_i = sum_{j!=i} 0.4 M_ij R_j       (scales DVE-TS, adds TT)
            #   arg = sum Q_i E_i ; th = tanh(arg)  (tanh ACT)
            #   merged = sum_r w_r R_r              (tree: ACT scales + DVE TT
            #                                        + GPSIMD STT links r5,r6)
            #   L = merged + s*th ; FT tiles = PE transpose(L)
            # Row-tile-outer loop; after each row tile i, fire sub-AllToAll i.
            with (
                tc.tile_pool(name="rstr", bufs=2) as prr,
                tc.tile_pool(name="gstr", bufs=2) as pg_p,
                tc.tile_pool(name="estr", bufs=2) as pe_p,
                tc.tile_pool(name="mstr", bufs=2) as pm_,
                tc.tile_pool(name="tstr", bufs=2) as ptn,
                tc.tile_pool(name="strpsum", bufs=4, space="PSUM") as sps,
            ):
                for i in range(NT):
                    for q in range(NCH):
                        c0 = q * C
                        rb = []
                        for j in range(NREL):
                            rj = prr.tile([P, C], BF16, tag=f"r{j}")
                            nc.sync.dma_start(
                                out=rj[:],
                                in_=a_strip[j, i * P:(i + 1) * P, c0:c0 + C])
                            rb.append(rj)

                        # --- interaction ---
                        # g_j = 1(R_j>0): DVE TS is_gt (4x mode)
                        gb = []
                        for j in range(3):
                            gj = pg_p.tile([P, C], BF16, tag=f"g{j}")
                            nc.vector.tensor_scalar(gj[:], rb[j][:], 0.0, None,
                                                    OP.is_gt)
                            gb.append(gj)
                        # Q_j = 1.5 R_j + g_j: scales on ACT (q0,q1) / DVE (q2);
                        # adds on DVE (Q0,Q1) / GPSIMD (Q2)
                        qq = []
                        for j in range(2):
                            qj = pg_p.tile([P, C], BF16, tag=f"q{j}")
                            nc.scalar.activation(qj[:], rb[j][:], AF.Copy,
                                                 scale=1.5)
                            nc.vector.tensor_tensor(qj[:], qj[:], gb[j][:],
                                                    OP.add)
                            qq.append(qj)
                        q2 = pg_p.tile([P, C], BF16, tag="q2")
                        nc.vector.tensor_scalar(q2[:], rb[2][:], 1.5, None,
                                                OP.mult)
                        nc.gpsimd.tensor_tensor(q2[:], q2[:], gb[2][:], OP.add)
                        qq.append(q2)

                        # E_i = c_ia R_a + c_ib R_b  (c = 0.4*M per-partition)
                        ee = []
                        for j in range(3):
                            o1, o2 = [x for x in range(3) if x != j]
                            ej = pe_p.tile([P, C], BF16, tag=f"e{j}")
                            nc.vector.tensor_scalar(ej[:], rb[o1][:],
                                                    c04_ap(j, o1), None, OP.mult)
                            te = pe_p.tile([P, C], BF16, tag=f"te{j}")
                            nc.scalar.activation(te[:], rb[o2][:], AF.Copy,
                                                 scale=c04_ap(j, o2))
                            nc.vector.tensor_tensor(ej[:], ej[:], te[:], OP.add)
                            ee.append(ej)

                        # arg = sum_j Q_j*E_j ; p_1 on GPSIMD
                        nc.vector.tensor_tensor(ee[0][:], qq[0][:], ee[0][:],
                                                OP.mult)
                        nc.gpsimd.tensor_tensor(ee[1][:], qq[1][:], ee[1][:],
                                                OP.mult)
                        nc.vector.tensor_tensor(ee[2][:], qq[2][:], ee[2][:],
                                                OP.mult)
                        nc.vector.tensor_tensor(ee[0][:], ee[0][:], ee[1][:],
                                                OP.add)
                        nc.vector.tensor_tensor(ee[0][:], ee[0][:], ee[2][:],
                                                OP.add)
                        tT = ptn.tile([P, C], BF16, tag="tT")
                        nc.scalar.activation(tT[:], ee[0][:], AF.Tanh)

                        # --- merged = sum_r w_r R_r ---
                        # scales: v0,v1 DVE TS; v2..v6 ACT copy-scale.
                        # tree adds: a,d,e DVE; b,c,m GPSIMD.
                        v0 = pm_.tile([P, C], BF16, tag="v0")
                        nc.vector.tensor_scalar(v0[:], rb[0][:], w_ap(0), None,
                                                OP.mult)
                        v1 = pm_.tile([P, C], BF16, tag="v1")
                        nc.vector.tensor_scalar(v1[:], rb[1][:], w_ap(1), None,
                                                OP.mult)
                        vs = []
                        for r in range(2, NREL):
                            vr = pm_.tile([P, C], BF16, tag=f"v{r}")
                            nc.scalar.activation(vr[:], rb[r][:], AF.Copy,
                                                 scale=w_ap(r))
                            vs.append(vr)
                        va, vb, vc, vd, ve = vs
                        nc.vector.tensor_tensor(v0[:], v0[:], v1[:], OP.add)
                        nc.gpsimd.tensor_tensor(va[:], va[:], vb[:], OP.add)
                        nc.gpsimd.tensor_tensor(vc[:], vc[:], vd[:], OP.add)
                        nc.vector.tensor_tensor(v0[:], v0[:], va[:], OP.add)
                        nc.vector.tensor_tensor(v0[:], v0[:], vc[:], OP.add)
                        mrow = pm_.tile([P, C], BF16, tag="mrow")
                        nc.gpsimd.tensor_tensor(mrow[:], v0[:], ve[:], OP.add)

                        # send merged chunks to sub-exchange buffer i
                        for d in (2 * q, 2 * q + 1):
                            nc.sync.dma_start(
                                out=sendbuf[i, d * P:(d + 1) * P, :],
                                in_=mrow[:, d * DST - c0:d * DST - c0 + DST])

                        # FT tile = merged^T + s*tanh^T via PE accumulation
                        # (matmul lhsT^T @ I; s folded into sidentb rhs)
                        for t in range(C // P):
                            kt = c0 // P + t
                            fsl = FT[:, kt * ROWS + i * P:kt * ROWS + (i + 1) * P]
                            pt1 = sps.tile([P, P], F32, tag="pt1")
                            nc.tensor.matmul(pt1[:],
                                             lhsT=mrow[:, t * P:(t + 1) * P],
                                             rhs=identb[:], start=True,
                                             stop=False)
                            nc.tensor.matmul(pt1[:],
                                             lhsT=tT[:, t * P:(t + 1) * P],
                                             rhs=sidentb[:], start=False,
                                             stop=True)
                            if t % 2:
                                nc.scalar.activation(fsl, pt1[:], AF.Copy)
                            else:
                                nc.vector.tensor_copy(fsl, pt1[:])

                    # fire sub-exchange for this row tile (all dests ready)
                    nc.gpsimd.collective_compute(
                        "AllToAll", OP.bypass, replica_groups=groups,
                        ins=[sendbuf[i].opt()], outs=[recvbuf[i].opt()])

            # ---- phase 2: add received column blocks + GCN layers ----
            with (
                tc.tile_pool(name="post", bufs=1) as post,
                tc.tile_pool(name="rcv", bufs=4) as prc,
                tc.tile_pool(name="postpsum", bufs=3, space="PSUM") as pops,
                tc.tile_pool(name="uvpsum", bufs=1, space="PSUM") as puvp,
            ):
                # ---- struct branch (rank-7) ----
                encsb = post.tile([P, KT * NREL], F32)
                for kt in range(KT):
                    nc.sync.dma_start(out=encsb[:, kt * NREL:(kt + 1) * NREL],
                                      in_=encode[kt * P:(kt + 1) * P, :])
                encb = post.tile([P, KT * NREL], BF16)
                nc.vector.tensor_copy(encb[:], encsb[:])
                encT = post.tile([NREL, N], BF16)
                for kt in range(KT):
                    pte = pops.tile([P, P], BF16, tag="pp_tr")
                    nc.tensor.transpose(pte[:NREL, :],
                                        encb[:, kt * NREL:(kt + 1) * NREL],
                                        identb[:])
                    nc.scalar.activation(encT[:, kt * P:(kt + 1) * P],
                                         pte[:NREL, :], AF.Copy)
                encRsb = post.tile([P, NT * NREL], F32)
                for i in range(NT):
                    nc.sync.dma_start(out=encRsb[:, i * NREL:(i + 1) * NREL],
                                      in_=enc_rows[i * P:(i + 1) * P, :])
                encRb = post.tile([P, NT * NREL], BF16)
                nc.vector.tensor_copy(encRb[:], encRsb[:])
                encRT = post.tile([NREL, ROWS], BF16)
                for i in range(NT):
                    pte = pops.tile([P, P], BF16, tag="pp_tr")
                    nc.tensor.transpose(pte[:NREL, :],
                                        encRb[:, i * NREL:(i + 1) * NREL],
                                        identb[:])
                    nc.scalar.activation(encRT[:, i * P:(i + 1) * P],
                                         pte[:NREL, :], AF.Copy)

                # H1 = encode^T @ Y1, scaled by sw
                ph = pops.tile([NREL, OUT], F32, tag="pp_mm")
                for kt in range(KT):
                    nc.tensor.matmul(ph[:],
                                     lhsT=encb[:, kt * NREL:(kt + 1) * NREL],
                                     rhs=YG[:, kt * 2 * OUT:kt * 2 * OUT + OUT],
                                     start=(kt == 0), stop=(kt == KT - 1))
                H1p = post.tile([NREL, OUT], BF16)
                nc.scalar.activation(H1p[:], ph[:], AF.Copy, scale=swt[:])

                # U3 = encode @ H1p + b1 (full, replicated)
                U3sb = post.tile([P, KT * OUT], BF16)
                for kt in range(KT):
                    pm3 = pops.tile([P, OUT], F32, tag="pp_mm")
                    nc.tensor.matmul(pm3[:], lhsT=encT[:, kt * P:(kt + 1) * P],
                                     rhs=H1p[:], start=True, stop=True)
                    nc.vector.tensor_tensor(U3sb[:, kt * OUT:(kt + 1) * OUT],
                                            pm3[:], b1b[:], OP.add)

                # G2 = (encode^T @ U3) @ W2, scaled by sw
                pg = pops.tile([NREL, OUT], F32, tag="pp_mm")
                for kt in range(KT):
                    nc.tensor.matmul(pg[:],
                                     lhsT=encb[:, kt * NREL:(kt + 1) * NREL],
                                     rhs=U3sb[:, kt * OUT:(kt + 1) * OUT],
                                     start=(kt == 0), stop=(kt == KT - 1))
                Gsb = post.tile([NREL, OUT], BF16)
                nc.vector.tensor_copy(Gsb[:], pg[:])
                pgt = pops.tile([OUT, NREL], BF16, tag="pp_tr")
                nc.tensor.transpose(pgt[:], Gsb[:], identb[:NREL, :NREL])
                GT = post.tile([OUT, NREL], BF16)
                nc.vector.tensor_copy(GT[:], pgt[:])
                pg2 = pops.tile([NREL, OUT], F32, tag="pp_mm")
                nc.tensor.matmul(pg2[:], lhsT=GT[:], rhs=W2b[:],
                                 start=True, stop=True)
                G2p = post.tile([NREL, OUT], BF16)
                nc.scalar.activation(G2p[:], pg2[:], AF.Copy, scale=swt[:])

                # U4 rows = enc_rows @ G2p + b2
                U4sb = post.tile([P, NT * OUT], F32)
                for i in range(NT):
                    pm4 = pops.tile([P, OUT], F32, tag="pp_mm")
                    nc.tensor.matmul(pm4[:], lhsT=encRT[:, i * P:(i + 1) * P],
                                     rhs=G2p[:], start=True, stop=True)
                    nc.vector.tensor_tensor(U4sb[:, i * OUT:(i + 1) * OUT],
                                            pm4[:], b2b[:], OP.add)

                # recv adds: FT[kt] += recv block (kt = d*NT + i), DVE/GPS split
                for i in range(NT):
                    for d in range(NCORES):
                        kt = d * NT + i
                        rt = prc.tile([P, DST], BF16, tag="rt")
                        nc.sync.dma_start(out=rt[:],
                                          in_=recvbuf[i, d * P:(d + 1) * P, :])
                        fsl = FT[:, kt * ROWS:(kt + 1) * ROWS]
                        if d % 4 == 3:
                            nc.gpsimd.tensor_tensor(fsl, fsl, rt[:], OP.add)
                        else:
                            nc.vector.tensor_tensor(fsl, fsl, rt[:], OP.add)

                # ---- fused [U1 | V]^T = ([Y1|G]^T @ final_A^T), 512-wide rhs,
                # single PSUM bank accumulation; i=3 k-tiles last (their recv
                # blocks arrive with sub-AllToAll 3 at phase-1 end).
                puv = puvp.tile([P, ROWS], F32, tag="uv")
                order = ([kt for kt in range(KT) if kt % NT != NT - 1]
                         + [kt for kt in range(KT) if kt % NT == NT - 1])
                for n, kt in enumerate(order):
                    nc.tensor.matmul(
                        puv[:],
                        lhsT=YG[:, kt * 2 * OUT:(kt + 1) * 2 * OUT],
                        rhs=FT[:, kt * ROWS:(kt + 1) * ROWS],
                        start=(n == 0), stop=(n == KT - 1))
                uvT = post.tile([P, ROWS], BF16)
                nc.vector.tensor_copy(uvT[:], puv[:])
                U1sb = post.tile([P, NT * OUT], F32)
                for i in range(NT):
                    ptb = pops.tile([P, P], BF16, tag="pp_tr")
                    nc.tensor.transpose(ptb[:], uvT[:, i * P:(i + 1) * P],
                                        identb[:])
                    vt = post.tile([P, OUT], BF16, tag="vt", bufs=2)
                    nc.vector.tensor_copy(vt[:], ptb[:, OUT:2 * OUT])
                    nc.sync.dma_start(out=agin[i * P:(i + 1) * P, :], in_=vt[:])
                    nc.vector.tensor_tensor(U1sb[:, i * OUT:(i + 1) * OUT],
                                            ptb[:, 0:OUT], b1b[:], OP.add)
                nc.gpsimd.collective_compute(
                    "AllGather", OP.bypass, replica_groups=groups,
                    ins=[agin[:].opt()], outs=[agout[:].opt()])

                # ---- Y2' = V_full + b1@W2 (bf16) ----
                Y2 = post.tile([P, KT * OUT], BF16)
                for kt in range(KT):
                    vtk = post.tile([P, OUT], BF16, tag="vtk", bufs=4)
                    nc.sync.dma_start(out=vtk[:],
                                      in_=agout[kt * P:(kt + 1) * P, :])
                    nc.vector.tensor_tensor(Y2[:, kt * OUT:(kt + 1) * OUT],
                                            vtk[:], hbb[:], OP.add)

                # ---- layer 2 transposed: U2^T = Y2^T @ final_A^T ----
                pu2 = puvp.tile([OUT, ROWS], F32, tag="u2")
                for kt in range(KT):
                    nc.tensor.matmul(
                        pu2[:],
                        lhsT=Y2[:, kt * OUT:(kt + 1) * OUT],
                        rhs=FT[:, kt * ROWS:(kt + 1) * ROWS],
                        start=(kt == 0), stop=(kt == KT - 1))
                u2T = post.tile([OUT, ROWS], BF16)
                nc.vector.tensor_copy(u2T[:], pu2[:])
                U2sb = post.tile([P, NT * OUT], F32)
                for i in range(NT):
                    pt2 = pops.tile([P, P], BF16, tag="pp_tr")
                    nc.tensor.transpose(pt2[:, :OUT], u2T[:, i * P:(i + 1) * P],
                                        identb[:OUT, :OUT])
                    nc.vector.tensor_tensor(U2sb[:, i * OUT:(i + 1) * OUT],
                                            pt2[:, 0:OUT], b2b[:], OP.add)

                # ---- combine + normalize + store ----
                for i in range(NT):
                    sl = slice(i * OUT, (i + 1) * OUT)
                    br1 = post.tile([P, OUT], F32, tag="br1", bufs=2)
                    nc.vector.tensor_tensor(br1[:], U1sb[:, sl], U2sb[:, sl],
                                            OP.add)
                    nc.vector.tensor_scalar(br1[:], br1[:], 0.5, None, OP.mult)
                    res = post.tile([P, OUT], F32, tag="res", bufs=2)
                    nc.vector.tensor_tensor(res[:], br1[:], U4sb[:, sl], OP.add)
                    nc.vector.tensor_scalar(res[:], res[:], 0.5, None, OP.mult)
                    _normalize(nc, post, pops, res, o_res, i)
                    _normalize(nc, post, pops, br1, o_b1, i)
                    u4 = post.tile([P, OUT], F32, tag="u4n", bufs=2)
                    nc.vector.tensor_copy(u4[:], U4sb[:, sl])
                    _normalize(nc, post, pops, u4, o_b2, i)

    _split_multi_waits(nc)
    return nc


_NC_CACHE = None


def get_nc():
    global _NC_CACHE
    if _NC_CACHE is None:
        _NC_CACHE = build_nc()
    return _NC_CACHE


def make_in_maps(feature, A_stack, encode, W1, b1, W2, b2, weight_b,
                 relation_interaction, interaction_strength, struct_weight):
    f32 = lambda x: np.ascontiguousarray(np.asarray(x, dtype=np.float32))
    featT = f32(np.asarray(feature, np.float32).T)
    enc = f32(encode)
    common = dict(
        featT=featT,
        encode=enc,
        W1=f32(W1),
        W2=f32(W2),
        b1=f32(np.reshape(b1, (1, OUT))),
        b2=f32(np.reshape(b2, (1, OUT))),
        wb=f32(np.reshape(np.asarray(weight_b, np.float32)[:, 0], (1, NREL))),
        ri=f32(np.reshape(relation_interaction, (1, 9))),
        s_=f32(np.reshape(interaction_strength, (1, 1))),
        sw=f32(np.reshape(struct_weight, (NREL, 1))),
    )
    in_maps = []
    import ml_dtypes
    A = np.asarray(A_stack, np.float32).astype(ml_dtypes.bfloat16)
    for c in range(NCORES):
        rows = slice(c * ROWS, (c + 1) * ROWS)
        m = dict(common)
        m["a_strip"] = np.ascontiguousarray(A[:, rows, :])
        m["enc_rows"] = f32(enc[rows])
        in_maps.append(m)
    return in_maps


def run(inputs, trace=False, tmpdir=None):
    nc = get_nc()
    in_maps = make_in_maps(**inputs)
    kres = run_bass_kernel_spmd(nc, in_maps, list(range(NCORES)),
                                trace=trace, tmpdir=tmpdir)
    res = kres.results
    result = np.concatenate([res[c]["o_res"] for c in range(NCORES)], axis=0)
    branch1 = np.concatenate([res[c]["o_b1"] for c in range(NCORES)], axis=0)
    branch2 = np.concatenate([res[c]["o_b2"] for c in range(NCORES)], axis=0)
    return (result, branch1, branch2), kres


def kernel(**inputs):
    return run(inputs)[0]


# revision 15
# speedup vs baseline: 1.2441x; 1.2441x over previous
"""MHGCN kernel for 8 Trainium2 NeuronCores.

Strategy: row-shard the [7,4096,4096] A_stack across 8 cores (512 rows each,
read once ~29.4MB/core bf16).  Each core streams its strip and builds the
transposed row-block of final_A:
    FT[k, m] = merged[m, k] + merged[k, m] + s*tanh_interaction[m, k]
 - local terms (merged[m,k], interaction) computed elementwise, balanced
   across DVE / ACT / GPSIMD using 4x-mode tensor_scalar ops + TT trees.
 - merged[k, m] column blocks arrive via FOUR sub-AllToAlls (one per local
   row-tile), fired as each row-tile finishes so 3 of 4 overlap phase 1.
Interaction identity:  arg = sum_i Q_i * E_i,  Q_i = 1.5 R_i + 1(R_i>0),
E_i = sum_{j!=i} 0.4 M_ij R_j;  interaction = tanh(arg).
Both GCN layers are then FT^T @ Y matmuls; V=final_A@G is AllGathered
between layers.  struct_adj = (encode*sw) @ encode^T is rank-7 and computed
locally without materializing it.
"""
import sys

sys.path.insert(0, "/opt/trn_rl_repo")

import numpy as np

import bass_rust
import concourse.bass as bass
import concourse.tile as tile
from concourse import mybir
from concourse.bass_utils import run_bass_kernel_spmd
from concourse.masks import make_identity
from concourse.vector_clock import ScopedClock

F32 = mybir.dt.float32
BF16 = mybir.dt.bfloat16
AF = mybir.ActivationFunctionType
OP = mybir.AluOpType

P = 128
N = 4096
NFEAT = 128
OUT = 64
NREL = 7
NCORES = 8
ROWS = N // NCORES        # 512 rows per core
NT = ROWS // P            # 4 row tiles per core
KT = N // P               # 32 k tiles
C = 1024                  # streaming column chunk
NCH = N // C              # 4 chunks per row tile
DST = ROWS                # exchange block width (512)


def _patched_drain_and_barrier(self, tick_clock, wait_clock):
    # Stock Tile attaches every outstanding proc's sem wait to one Drain;
    # this walrus build caps sync waits per instruction, so split them
    # into single-wait drains.
    drain_inst = self.nc.sync.drain()
    wait_clock.add_sem_waits(
        drain_inst.ins, ScopedClock({None: tick_clock.global_clock})
    )
    si = drain_inst.ins.sync_info
    if si is not None and len(si.on_wait) > 1:
        waits = list(si.on_wait)
        si.on_wait = [waits[0]]
        for w in waits[1:]:
            extra = self.nc.sync.drain()
            extra.ins.sync_info = bass_rust.SyncInfo(on_wait=[w], on_update=[])
    self.nc.all_engine_barrier()
    assert self.sems is not None
    popped = self.nc._tile_sem_poison_stack.pop()
    assert popped is self._sem_poison
    self.nc.clear_and_free_semaphores(list(self.sems.allocated().values()))
    self.nc.all_engine_barrier()


tile.TileContext._drain_and_barrier = _patched_drain_and_barrier


def _split_multi_waits(nc, limit=1):
    """Walrus in this container caps sync-wait commands per instruction.
    Hoist all-but-`limit` waits of any instruction onto single-wait NoOps
    inserted just before it on the same engine queue."""
    cnt = 0
    for fn in nc.m.functions:
        for blk in fn.blocks:
            lst = list(blk.instructions)
            out = []
            changed = False
            for inst in lst:
                si = inst.sync_info
                if si is not None and len(si.on_wait) > limit:
                    waits = list(si.on_wait)
                    for w in waits[:-limit]:
                        n = bass_rust.InstNoOp(name=f"wsplit-{cnt}")
                        cnt += 1
                        n.engine = inst.engine
                        n.bass_nofuse = True
                        n.sync_info = bass_rust.SyncInfo(on_wait=[w],
                                                         on_update=[])
                        nc.register_instruction(n, overwrite=True)
                        out.append(n)
                    si.on_wait = waits[-limit:]
                    changed = True
                out.append(inst)
            if changed:
                blk.instructions = out
    return cnt


def _normalize(nc, pool, psum, x, out_dram, i):
    """l2-normalize rows of x [P, OUT] and DMA to out_dram[i*P:(i+1)*P]."""
    sq = pool.tile([P, OUT], F32, tag="nrm_sq")
    nrm = pool.tile([P, 1], F32, tag="nrm_n")
    nc.vector.tensor_tensor(sq[:], x[:], x[:], OP.mult)
    nc.vector.tensor_reduce(nrm[:], sq[:], mybir.AxisListType.X, OP.add)
    nr = pool.tile([P, 1], F32, tag="nrm_r")
    nc.scalar.activation(nr[:], nrm[:], AF.Sqrt)
    nc.vector.tensor_scalar(nr[:], nr[:], 1e-12, None, OP.max)
    ninv = pool.tile([P, 1], F32, tag="nrm_i")
    nc.vector.reciprocal(ninv[:], nr[:])
    y = pool.tile([P, OUT], F32, tag="nrm_y")
    nc.vector.tensor_scalar(y[:], x[:], ninv[:], None, OP.mult)
    nc.sync.dma_start(out=out_dram[i * P:(i + 1) * P, :], in_=y[:])


def build_nc():
    nc = bass.Bass()

    a_strip = nc.dram_tensor("a_strip", [NREL, ROWS, N], BF16, kind="ExternalInput")
    featT = nc.dram_tensor("featT", [NFEAT, N], F32, kind="ExternalInput")
    encode = nc.dram_tensor("encode", [N, NREL], F32, kind="ExternalInput")
    enc_rows = nc.dram_tensor("enc_rows", [ROWS, NREL], F32, kind="ExternalInput")
    W1 = nc.dram_tensor("W1", [NFEAT, OUT], F32, kind="ExternalInput")
    W2 = nc.dram_tensor("W2", [OUT, OUT], F32, kind="ExternalInput")
    b1 = nc.dram_tensor("b1", [1, OUT], F32, kind="ExternalInput")
    b2 = nc.dram_tensor("b2", [1, OUT], F32, kind="ExternalInput")
    wb = nc.dram_tensor("wb", [1, NREL], F32, kind="ExternalInput")
    ri = nc.dram_tensor("ri", [1, 9], F32, kind="ExternalInput")
    s_ = nc.dram_tensor("s_", [1, 1], F32, kind="ExternalInput")
    sw = nc.dram_tensor("sw", [NREL, 1], F32, kind="ExternalInput")

    o_res = nc.dram_tensor("o_res", [ROWS, OUT], F32, kind="ExternalOutput")
    o_b1 = nc.dram_tensor("o_b1", [ROWS, OUT], F32, kind="ExternalOutput")
    o_b2 = nc.dram_tensor("o_b2", [ROWS, OUT], F32, kind="ExternalOutput")

    groups = [list(range(NCORES))]

    with tile.TileContext(nc) as tc:
        with (
            tc.tile_pool(name="persist", bufs=1) as pp,
            tc.tile_pool(name="dram", bufs=1, space="DRAM") as dpool,
        ):
            # ---- constants / small tensors ----
            ident = pp.tile([P, P], F32)
            make_identity(nc, ident)
            identb = pp.tile([P, P], BF16)
            nc.vector.tensor_copy(identb[:], ident[:])

            ones_1p = pp.tile([1, P], F32)
            nc.vector.memset(ones_1p[:], 1.0)

            # scalar staging: [0:7]=w_r, [7:16]=M flat, [16]=s
            sstage = pp.tile([1, 17], F32)
            nc.sync.dma_start(out=sstage[:, 0:NREL], in_=wb[:])
            nc.sync.dma_start(out=sstage[:, NREL:NREL + 9], in_=ri[:])
            nc.sync.dma_start(out=sstage[:, 16:17], in_=s_[:])

            W1t = pp.tile([NFEAT, OUT], F32)
            nc.sync.dma_start(out=W1t[:], in_=W1[:])
            W2t = pp.tile([OUT, OUT], F32)
            nc.sync.dma_start(out=W2t[:], in_=W2[:])
            b1st = pp.tile([1, OUT], F32)
            nc.sync.dma_start(out=b1st[:], in_=b1[:])
            b2st = pp.tile([1, OUT], F32)
            nc.sync.dma_start(out=b2st[:], in_=b2[:])
            swt = pp.tile([NREL, 1], F32)
            nc.sync.dma_start(out=swt[:], in_=sw[:])

            scal = pp.tile([P, 17], F32)
            b1b = pp.tile([P, OUT], F32)
            b2b = pp.tile([P, OUT], F32)
            with tc.tile_pool(name="ppsum", bufs=1, space="PSUM") as pps:
                pb = pps.tile([P, 17], F32, tag="pb")
                nc.tensor.matmul(pb[:], lhsT=ones_1p[:], rhs=sstage[:],
                                 start=True, stop=True)
                nc.vector.tensor_copy(scal[:], pb[:])
                pb1 = pps.tile([P, OUT], F32, tag="pb1")
                nc.tensor.matmul(pb1[:], lhsT=ones_1p[:], rhs=b1st[:],
                                 start=True, stop=True)
                nc.vector.tensor_copy(b1b[:], pb1[:])
                pb2 = pps.tile([P, OUT], F32, tag="pb2")
                nc.tensor.matmul(pb2[:], lhsT=ones_1p[:], rhs=b2st[:],
                                 start=True, stop=True)
                nc.vector.tensor_copy(b2b[:], pb2[:])

            # fp32 broadcast scalars (TensorScalarPtr requires fp32 scalars)
            scal04 = pp.tile([P, 9], F32)
            nc.vector.tensor_scalar(scal04[:], scal[:, NREL:NREL + 9], 0.4,
                                    None, OP.mult)
            # s-scaled identity: FT transpose-accumulate applies s*tanh via rhs
            sidentb = pp.tile([P, P], BF16)
            nc.vector.tensor_scalar(sidentb[:], identb[:], scal[:, 16:17],
                                    None, OP.mult)

            def w_ap(r):
                return scal[:, r:r + 1]

            s_ap = scal[:, 16:17]

            def c04_ap(i, j):
                return scal04[:, 3 * i + j:3 * i + j + 1]

            # ---- persistent big tensors (bf16; PSUM accumulates fp32) ----
            FT = pp.tile([P, KT * ROWS], BF16)    # final_A^T: 32 k-tiles x [128, 512]
            YG = pp.tile([P, KT * 2 * OUT], BF16)  # [Y1 | G] per k-tile

            # ---- DRAM bounce buffers (4 sub-exchanges, one per row tile) ----
            sendbuf = dpool.tile([NT, NCORES * P, DST], BF16)
            recvbuf = dpool.tile([NT, NCORES * P, DST], BF16)
            agin = dpool.tile([ROWS, OUT], BF16)
            agout = dpool.tile([N, OUT], BF16, addr_space="Shared")

            # ---- prep: Y1 = feature @ W1 ----
            with (
                tc.tile_pool(name="prep", bufs=1) as prep,
                tc.tile_pool(name="preppsum", bufs=2, space="PSUM") as prps,
            ):
                ftile = prep.tile([NFEAT, N], F32)
                nc.sync.dma_start(out=ftile[:], in_=featT[:])
                fbf = prep.tile([NFEAT, N], BF16)
                nc.vector.tensor_copy(fbf[:], ftile[:])
                W1b = pp.tile([NFEAT, OUT], BF16)
                nc.vector.tensor_copy(W1b[:], W1t[:])
                W2b = pp.tile([OUT, OUT], BF16)
                nc.vector.tensor_copy(W2b[:], W2t[:])
                # W12 = W1 @ W2 (via W1^T transpose), h = b1 @ W2
                pw1t = prps.tile([P, P], BF16, tag="prsm")
                nc.tensor.transpose(pw1t[:OUT, :NFEAT], W1b[:], identb[:])
                W1T = prep.tile([OUT, NFEAT], BF16)
                nc.vector.tensor_copy(W1T[:], pw1t[:OUT, :NFEAT])
                pw12 = prps.tile([NFEAT, OUT], F32, tag="prsm")
                nc.tensor.matmul(pw12[:], lhsT=W1T[:], rhs=W2b[:],
                                 start=True, stop=True)
                W12b = pp.tile([NFEAT, OUT], BF16)
                nc.vector.tensor_copy(W12b[:], pw12[:])
                b1v = prep.tile([OUT, 1], BF16)
                pb1t = prps.tile([OUT, 1], BF16, tag="prsm")
                b1bf = prep.tile([1, OUT], BF16)
                nc.vector.tensor_copy(b1bf[:], b1st[:])
                nc.tensor.transpose(pb1t[:], b1bf[:], identb[:1, :1])
                nc.vector.tensor_copy(b1v[:], pb1t[:])
                phh = prps.tile([1, OUT], F32, tag="prsm")
                nc.tensor.matmul(phh[:], lhsT=b1v[:], rhs=W2b[:],
                                 start=True, stop=True)
                hst = prep.tile([1, OUT], F32)
                nc.vector.tensor_copy(hst[:], phh[:])
                phb = prps.tile([P, OUT], F32, tag="prsm")
                nc.tensor.matmul(phb[:], lhsT=ones_1p[:], rhs=hst[:],
                                 start=True, stop=True)
                hb = pp.tile([P, OUT], F32)
                nc.vector.tensor_copy(hb[:], phb[:])
                hbb = pp.tile([P, OUT], BF16)
                nc.vector.tensor_copy(hbb[:], hb[:])

                for kt in range(KT):
                    pm = prps.tile([P, OUT], F32, tag="y1p")
                    nc.tensor.matmul(pm[:], lhsT=fbf[:, kt * P:(kt + 1) * P],
                                     rhs=W1b[:], start=True, stop=True)
                    nc.vector.tensor_copy(
                        YG[:, kt * 2 * OUT:kt * 2 * OUT + OUT], pm[:])
                    pg_ = prps.tile([P, OUT], F32, tag="gp")
                    nc.tensor.matmul(pg_[:], lhsT=fbf[:, kt * P:(kt + 1) * P],
                                     rhs=W12b[:], start=True, stop=True)
                    nc.vector.tensor_copy(
                        YG[:, kt * 2 * OUT + OUT:(kt + 1) * 2 * OUT], pg_[:])

            # ---- phase 1: stream A row block ----
            # Per chunk [128, C]: R_j = relation j slice.
            #   g_j = 1(R_j > 0)                    DVE tensor_scalar is_gt (4x)
            #   Q_j = 1.5 R_j + g_j                 (Q2 on GPSIMD via STT)
            #   E_i = sum_{j!=i} 0.4 M_ij R_j       (scales DVE-TS, adds TT)
            #   arg = sum Q_i E_i ; th = tanh(arg)  (tanh ACT)
            #   merged = sum_r w_r R_r              (tree: ACT scales + DVE TT
            #                                        + GPSIMD STT links r5,r6)
            #   L = merged + s*th ; FT tiles = PE transpose(L)
            # Row-tile-outer loop; after each row tile i, fire sub-AllToAll i.
            with (
                tc.tile_pool(name="rstr", bufs=2) as prr,
                tc.tile_pool(name="gstr", bufs=2) as pg_p,
                tc.tile_pool(name="estr", bufs=2) as pe_p,
                tc.tile_pool(name="mstr", bufs=2) as pm_,
                tc.tile_pool(name="tstr", bufs=2) as ptn,
                tc.tile_pool(name="strpsum", bufs=4, space="PSUM") as sps,
            ):
                for i in range(NT):
                    for q in range(NCH):
                        c0 = q * C
                        rb = []
                        for j in range(NREL):
                            rj = prr.tile([P, C], BF16, tag=f"r{j}")
                            nc.sync.dma_start(
                                out=rj[:],
                                in_=a_strip[j, i * P:(i + 1) * P, c0:c0 + C])
                            rb.append(rj)

                        # --- interaction ---
                        # g_j = 1(R_j>0): DVE TS is_gt (4x mode)
                        gb = []
                        for j in range(3):
                            gj = pg_p.tile([P, C], BF16, tag=f"g{j}")
                            nc.vector.tensor_scalar(gj[:], rb[j][:], 0.0, None,
                                                    OP.is_gt)
                            gb.append(gj)
                        # Q_j = 1.5 R_j + g_j: scales on ACT (q0,q1) / DVE (q2);
                        # adds on DVE (Q0,Q1) / GPSIMD (Q2)
                        qq = []
                        for j in range(2):
                            qj = pg_p.tile([P, C], BF16, tag=f"q{j}")
                            nc.scalar.activation(qj[:], rb[j][:], AF.Copy,
                                                 scale=1.5)
                            nc.vector.tensor_tensor(qj[:], qj[:], gb[j][:],
                                                    OP.add)
                            qq.append(qj)
                        q2 = pg_p.tile([P, C], BF16, tag="q2")
                        nc.vector.tensor_scalar(q2[:], rb[2][:], 1.5, None,
                                                OP.mult)
                        nc.vector.tensor_tensor(q2[:], q2[:], gb[2][:], OP.add)
                        qq.append(q2)

                        # E_i = c_ia R_a + c_ib R_b  (c = 0.4*M per-partition)
                        ee = []
                        for j in range(3):
                            o1, o2 = [x for x in range(3) if x != j]
                            ej = pe_p.tile([P, C], BF16, tag=f"e{j}")
                            nc.vector.tensor_scalar(ej[:], rb[o1][:],
                                                    c04_ap(j, o1), None, OP.mult)
                            te = pe_p.tile([P, C], BF16, tag=f"te{j}")
                            nc.scalar.activation(te[:], rb[o2][:], AF.Copy,
                                                 scale=c04_ap(j, o2))
                            nc.vector.tensor_tensor(ej[:], ej[:], te[:], OP.add)
                            ee.append(ej)

                        # arg = sum_j Q_j*E_j ; p_1 on GPSIMD
                        nc.vector.tensor_tensor(ee[0][:], qq[0][:], ee[0][:],
                                                OP.mult)
                        nc.vector.tensor_tensor(ee[1][:], qq[1][:], ee[1][:],
                                                OP.mult)
                        nc.vector.tensor_tensor(ee[2][:], qq[2][:], ee[2][:],
                                                OP.mult)
                        nc.vector.tensor_tensor(ee[0][:], ee[0][:], ee[1][:],
                                                OP.add)
                        nc.vector.tensor_tensor(ee[0][:], ee[0][:], ee[2][:],
                                                OP.add)
                        tT = ptn.tile([P, C], BF16, tag="tT")
                        nc.scalar.activation(tT[:], ee[0][:], AF.Tanh)

                        # --- merged = sum_r w_r R_r ---
                        # scales: v0,v1 DVE TS; v2..v6 ACT copy-scale.
                        # tree adds: a,d,e DVE; b,c,m GPSIMD.
                        v0 = pm_.tile([P, C], BF16, tag="v0")
                        nc.vector.tensor_scalar(v0[:], rb[0][:], w_ap(0), None,
                                                OP.mult)
                        v1 = pm_.tile([P, C], BF16, tag="v1")
                        nc.vector.tensor_scalar(v1[:], rb[1][:], w_ap(1), None,
                                                OP.mult)
                        vs = []
                        for r in range(2, NREL):
                            vr = pm_.tile([P, C], BF16, tag=f"v{r}")
                            nc.scalar.activation(vr[:], rb[r][:], AF.Copy,
                                                 scale=w_ap(r))
                            vs.append(vr)
                        va, vb, vc, vd, ve = vs
                        nc.vector.tensor_tensor(v0[:], v0[:], v1[:], OP.add)
                        nc.vector.tensor_tensor(va[:], va[:], vb[:], OP.add)
                        nc.vector.tensor_tensor(vc[:], vc[:], vd[:], OP.add)
                        nc.vector.tensor_tensor(v0[:], v0[:], va[:], OP.add)
                        nc.vector.tensor_tensor(v0[:], v0[:], vc[:], OP.add)
                        mrow = pm_.tile([P, C], BF16, tag="mrow")
                        nc.vector.tensor_tensor(mrow[:], v0[:], ve[:], OP.add)

                        # send merged chunks to sub-exchange buffer i
                        for d in (2 * q, 2 * q + 1):
                            nc.sync.dma_start(
                                out=sendbuf[i, d * P:(d + 1) * P, :],
                                in_=mrow[:, d * DST - c0:d * DST - c0 + DST])

                        # FT tile = merged^T + s*tanh^T via PE accumulation
                        # (matmul lhsT^T @ I; s folded into sidentb rhs)
                        for t in range(C // P):
                            kt = c0 // P + t
                            fsl = FT[:, kt * ROWS + i * P:kt * ROWS + (i + 1) * P]
                            pt1 = sps.tile([P, P], F32, tag="pt1")
                            nc.tensor.matmul(pt1[:],
                                             lhsT=mrow[:, t * P:(t + 1) * P],
                                             rhs=identb[:], start=True,
                                             stop=False)
                            nc.tensor.matmul(pt1[:],
                                             lhsT=tT[:, t * P:(t + 1) * P],
                                             rhs=sidentb[:], start=False,
                                             stop=True)
                            if t % 2:
                                nc.scalar.activation(fsl, pt1[:], AF.Copy)
                            else:
                                nc.vector.tensor_copy(fsl, pt1[:])

                    # fire sub-exchange for this row tile (all dests ready)
                    nc.gpsimd.collective_compute(
                        "AllToAll", OP.bypass, replica_groups=groups,
                        ins=[sendbuf[i].opt()], outs=[recvbuf[i].opt()])

            # ---- phase 2: add received column blocks + GCN layers ----
            with (
                tc.tile_pool(name="post", bufs=1) as post,
                tc.tile_pool(name="rcv", bufs=4) as prc,
                tc.tile_pool(name="postpsum", bufs=3, space="PSUM") as pops,
                tc.tile_pool(name="uvpsum", bufs=1, space="PSUM") as puvp,
            ):
                # recv adds: FT[kt] += recv block (kt = d*NT + i), DVE/GPS split
                for i in range(NT):
                    for d in range(NCORES):
                        kt = d * NT + i
                        rt = prc.tile([P, DST], BF16, tag="rt")
                        nc.sync.dma_start(out=rt[:],
                                          in_=recvbuf[i, d * P:(d + 1) * P, :])
                        fsl = FT[:, kt * ROWS:(kt + 1) * ROWS]
                        nc.vector.tensor_tensor(fsl, fsl, rt[:], OP.add)

                # ---- fused [U1 | V]^T = ([Y1|G]^T @ final_A^T), 512-wide rhs,
                # single PSUM bank accumulation; i=3 k-tiles last (their recv
                # blocks arrive with sub-AllToAll 3 at phase-1 end).
                puv = puvp.tile([P, ROWS], F32, tag="uv")
                order = ([kt for kt in range(KT) if kt % NT != NT - 1]
                         + [kt for kt in range(KT) if kt % NT == NT - 1])
                for n, kt in enumerate(order):
                    nc.tensor.matmul(
                        puv[:],
                        lhsT=YG[:, kt * 2 * OUT:(kt + 1) * 2 * OUT],
                        rhs=FT[:, kt * ROWS:(kt + 1) * ROWS],
                        start=(n == 0), stop=(n == KT - 1))
                uvT = post.tile([P, ROWS], BF16)
                nc.vector.tensor_copy(uvT[:], puv[:])
                U1sb = post.tile([P, NT * OUT], F32)
                for i in range(NT):
                    ptb = pops.tile([P, P], BF16, tag="pp_tr")
                    nc.tensor.transpose(ptb[:], uvT[:, i * P:(i + 1) * P],
                                        identb[:])
                    vt = post.tile([P, OUT], BF16, tag="vt", bufs=2)
                    nc.vector.tensor_copy(vt[:], ptb[:, OUT:2 * OUT])
                    nc.sync.dma_start(out=agin[i * P:(i + 1) * P, :], in_=vt[:])
                    nc.vector.tensor_tensor(U1sb[:, i * OUT:(i + 1) * OUT],
                                            ptb[:, 0:OUT], b1b[:], OP.add)
                nc.gpsimd.collective_compute(
                    "AllGather", OP.bypass, replica_groups=groups,
                    ins=[agin[:].opt()], outs=[agout[:].opt()])

                # ---- struct branch (rank-7) ----
                encsb = post.tile([P, KT * NREL], F32)
                for kt in range(KT):
                    nc.sync.dma_start(out=encsb[:, kt * NREL:(kt + 1) * NREL],
                                      in_=encode[kt * P:(kt + 1) * P, :])
                encb = post.tile([P, KT * NREL], BF16)
                nc.vector.tensor_copy(encb[:], encsb[:])
                encT = post.tile([NREL, N], BF16)
                for kt in range(KT):
                    pte = pops.tile([P, P], BF16, tag="pp_tr")
                    nc.tensor.transpose(pte[:NREL, :],
                                        encb[:, kt * NREL:(kt + 1) * NREL],
                                        identb[:])
                    nc.scalar.activation(encT[:, kt * P:(kt + 1) * P],
                                         pte[:NREL, :], AF.Copy)
                encRsb = post.tile([P, NT * NREL], F32)
                for i in range(NT):
                    nc.sync.dma_start(out=encRsb[:, i * NREL:(i + 1) * NREL],
                                      in_=enc_rows[i * P:(i + 1) * P, :])
                encRb = post.tile([P, NT * NREL], BF16)
                nc.vector.tensor_copy(encRb[:], encRsb[:])
                encRT = post.tile([NREL, ROWS], BF16)
                for i in range(NT):
                    pte = pops.tile([P, P], BF16, tag="pp_tr")
                    nc.tensor.transpose(pte[:NREL, :],
                                        encRb[:, i * NREL:(i + 1) * NREL],
                                        identb[:])
                    nc.scalar.activation(encRT[:, i * P:(i + 1) * P],
                                         pte[:NREL, :], AF.Copy)

                # H1 = encode^T @ Y1, scaled by sw
                ph = pops.tile([NREL, OUT], F32, tag="pp_mm")
                for kt in range(KT):
                    nc.tensor.matmul(ph[:],
                                     lhsT=encb[:, kt * NREL:(kt + 1) * NREL],
                                     rhs=YG[:, kt * 2 * OUT:kt * 2 * OUT + OUT],
                                     start=(kt == 0), stop=(kt == KT - 1))
                H1p = post.tile([NREL, OUT], BF16)
                nc.scalar.activation(H1p[:], ph[:], AF.Copy, scale=swt[:])

                # U3 = encode @ H1p + b1 (full, replicated)
                U3sb = post.tile([P, KT * OUT], BF16)
                for kt in range(KT):
                    pm3 = pops.tile([P, OUT], F32, tag="pp_mm")
                    nc.tensor.matmul(pm3[:], lhsT=encT[:, kt * P:(kt + 1) * P],
                                     rhs=H1p[:], start=True, stop=True)
                    nc.vector.tensor_tensor(U3sb[:, kt * OUT:(kt + 1) * OUT],
                                            pm3[:], b1b[:], OP.add)

                # G2 = (encode^T @ U3) @ W2, scaled by sw
                pg = pops.tile([NREL, OUT], F32, tag="pp_mm")
                for kt in range(KT):
                    nc.tensor.matmul(pg[:],
                                     lhsT=encb[:, kt * NREL:(kt + 1) * NREL],
                                     rhs=U3sb[:, kt * OUT:(kt + 1) * OUT],
                                     start=(kt == 0), stop=(kt == KT - 1))
                Gsb = post.tile([NREL, OUT], BF16)
                nc.vector.tensor_copy(Gsb[:], pg[:])
                pgt = pops.tile([OUT, NREL], BF16, tag="pp_tr")
                nc.tensor.transpose(pgt[:], Gsb[:], identb[:NREL, :NREL])
                GT = post.tile([OUT, NREL], BF16)
                nc.vector.tensor_copy(GT[:], pgt[:])
                pg2 = pops.tile([NREL, OUT], F32, tag="pp_mm")
                nc.tensor.matmul(pg2[:], lhsT=GT[:], rhs=W2b[:],
                                 start=True, stop=True)
                G2p = post.tile([NREL, OUT], BF16)
                nc.scalar.activation(G2p[:], pg2[:], AF.Copy, scale=swt[:])

                # U4 rows = enc_rows @ G2p + b2
                U4sb = post.tile([P, NT * OUT], F32)
                for i in range(NT):
                    pm4 = pops.tile([P, OUT], F32, tag="pp_mm")
                    nc.tensor.matmul(pm4[:], lhsT=encRT[:, i * P:(i + 1) * P],
                                     rhs=G2p[:], start=True, stop=True)
                    nc.vector.tensor_tensor(U4sb[:, i * OUT:(i + 1) * OUT],
                                            pm4[:], b2b[:], OP.add)


                # ---- Y2' = V_full + b1@W2 (bf16) ----
                Y2 = post.tile([P, KT * OUT], BF16)
                for kt in range(KT):
                    vtk = post.tile([P, OUT], BF16, tag="vtk", bufs=4)
                    nc.sync.dma_start(out=vtk[:],
                                      in_=agout[kt * P:(kt + 1) * P, :])
                    nc.vector.tensor_tensor(Y2[:, kt * OUT:(kt + 1) * OUT],
                                            vtk[:], hbb[:], OP.add)

                # ---- layer 2 transposed: U2^T = Y2^T @ final_A^T ----
                pu2 = puvp.tile([OUT, ROWS], F32, tag="u2")
                for kt in range(KT):
                    nc.tensor.matmul(
                        pu2[:],
                        lhsT=Y2[:, kt * OUT:(kt + 1) * OUT],
                        rhs=FT[:, kt * ROWS:(kt + 1) * ROWS],
                        start=(kt == 0), stop=(kt == KT - 1))
                u2T = post.tile([OUT, ROWS], BF16)
                nc.vector.tensor_copy(u2T[:], pu2[:])
                U2sb = post.tile([P, NT * OUT], F32)
                for i in range(NT):
                    pt2 = pops.tile([P, P], BF16, tag="pp_tr")
                    nc.tensor.transpose(pt2[:, :OUT], u2T[:, i * P:(i + 1) * P],
                                        identb[:OUT, :OUT])
                    nc.vector.tensor_tensor(U2sb[:, i * OUT:(i + 1) * OUT],
                                            pt2[:, 0:OUT], b2b[:], OP.add)

                # ---- combine + normalize + store ----
                for i in range(NT):
                    sl = slice(i * OUT, (i + 1) * OUT)
                    br1 = post.tile([P, OUT], F32, tag="br1", bufs=2)
                    nc.vector.tensor_tensor(br1[:], U1sb[:, sl], U2sb[:, sl],
                                            OP.add)
                    nc.vector.tensor_scalar(br1[:], br1[:], 0.5, None, OP.mult)
                    res = post.tile([P, OUT], F32, tag="res", bufs=2)
                    nc.vector.tensor_tensor(res[:], br1[:], U4sb[:, sl], OP.add)
                    nc.vector.tensor_scalar(res[:], res[:], 0.5, None, OP.mult)
                    _normalize(nc, post, pops, res, o_res, i)
                    _normalize(nc, post, pops, br1, o_b1, i)
                    u4 = post.tile([P, OUT], F32, tag="u4n", bufs=2)
                    nc.vector.tensor_copy(u4[:], U4sb[:, sl])
                    _normalize(nc, post, pops, u4, o_b2, i)

    _split_multi_waits(nc)
    return nc


_NC_CACHE = None


def get_nc():
    global _NC_CACHE
    if _NC_CACHE is None:
        _NC_CACHE = build_nc()
    return _NC_CACHE


def make_in_maps(feature, A_stack, encode, W1, b1, W2, b2, weight_b,
                 relation_interaction, interaction_strength, struct_weight):
    f32 = lambda x: np.ascontiguousarray(np.asarray(x, dtype=np.float32))
    featT = f32(np.asarray(feature, np.float32).T)
    enc = f32(encode)
    common = dict(
        featT=featT,
        encode=enc,
        W1=f32(W1),
        W2=f32(W2),
        b1=f32(np.reshape(b1, (1, OUT))),
        b2=f32(np.reshape(b2, (1, OUT))),
        wb=f32(np.reshape(np.asarray(weight_b, np.float32)[:, 0], (1, NREL))),
        ri=f32(np.reshape(relation_interaction, (1, 9))),
        s_=f32(np.reshape(interaction_strength, (1, 1))),
        sw=f32(np.reshape(struct_weight, (NREL, 1))),
    )
    in_maps = []
    import ml_dtypes
    A = np.asarray(A_stack, np.float32).astype(ml_dtypes.bfloat16)
    for c in range(NCORES):
        rows = slice(c * ROWS, (c + 1) * ROWS)
        m = dict(common)
        m["a_strip"] = np.ascontiguousarray(A[:, rows, :])
        m["enc_rows"] = f32(enc[rows])
        in_maps.append(m)
    return in_maps


def run(inputs, trace=False, tmpdir=None):
    nc = get_nc()
    in_maps = make_in_maps(**inputs)
    kres = run_bass_kernel_spmd(nc, in_maps, list(range(NCORES)),
                                trace=trace, tmpdir=tmpdir)
    res = kres.results
    result = np.concatenate([res[c]["o_res"] for c in range(NCORES)], axis=0)
    branch1 = np.concatenate([res[c]["o_b1"] for c in range(NCORES)], axis=0)
    branch2 = np.concatenate([res[c]["o_b2"] for c in range(NCORES)], axis=0)
    return (result, branch1, branch2), kres


def kernel(**inputs):
    return run(inputs)[0]
